# revision 1
# baseline (speedup 1.0000x reference)
"""Multi-head self-attention (B=2, S=4096, H=512, 8 heads) on 8 NeuronCores.

Sharding: core c -> batch b=c//4, query block c%4 (1024 query rows).

v2: ACT(exp)-paced single pipeline. The softmax exp on ScalarE is the hard
floor (~0.26M free-dim elems/core @1.2GHz); everything else is arranged to
hide under it:
  - S^T matmuls for the two heads of a pair run CONCURRENTLY as 64x128 PE
    row-tiles (tile_position (0,0)/(64,0)), one 128-key chunk per step,
    written to adjacent PSUM banks so one [128,2,512] ACT instr covers both.
  - K/V (and Q) projections are streamed as jobs inside the qb0 attention
    loop instead of a serial phase, so the PE fills its ACT-idle gaps and
    HAM stays warm.
  - mask multiply: one DVE tensor_tensor per chunk with a stride-0
    broadcast AP sharing the mask slice across both heads (bf16 2x mode).
  - softmax denominators ride the PV matmul as a 65th V column; reciprocal
    via the fast custom-DVE approx op; 1/d broadcast across partitions with
    a tiny K=1 PE outer product.
  - PSUM budget (8 banks): s-tiles 2x[128,2,512] (4) + ctx/rb pool 3x[65,512]
    (3) + out-proj/proj scratch 1x[128,512] (1).
"""

import os
import sys
from collections import deque

import numpy as np

for _p in ("/opt/trn_rl_repo", "/root/.axon_site/_ro/trn_rl_repo"):
    if os.path.isdir(_p) and _p not in sys.path:
        sys.path.insert(0, _p)

import ml_dtypes
import concourse.bass as bass
import concourse.mybir as mybir
import concourse.tile as tile
from concourse.bass_utils import run_bass_kernel_spmd


def _ensure_axon_hooks():
    """bass_utils' trace path imports antenv.axon_hooks, which this image
    lacks; register a functional stand-in so tracing works when available
    and degrades cleanly otherwise."""
    import types

    if "antenv.axon_hooks" in sys.modules:
        return
    try:
        import antenv
    except ImportError:
        return
    mod = types.ModuleType("antenv.axon_hooks")
    _hook = [None]
    mod.set_axon_ntff_profile_hook = lambda h: _hook.__setitem__(0, h)
    mod.get_axon_ntff_profile_hook = lambda: _hook[0]
    sys.modules["antenv.axon_hooks"] = mod
    antenv.axon_hooks = mod
    try:
        from trn_agent_boot.trn_boot import _ntff_profile_via_ctypes

        hook = _ntff_profile_via_ctypes("/opt/axon/libaxon_pjrt.so")
        mod.set_axon_ntff_profile_hook(hook)
    except Exception:
        pass


_ensure_axon_hooks()


def _ensure_axon_hooks():
    """bass_utils' trace path imports antenv.axon_hooks, which this image
    lacks; register a functional stand-in so tracing works when available
    and degrades cleanly otherwise."""
    import types

    if "antenv.axon_hooks" in sys.modules:
        return
    try:
        import antenv
    except ImportError:
        return
    mod = types.ModuleType("antenv.axon_hooks")
    _hook = [None]
    mod.set_axon_ntff_profile_hook = lambda h: _hook.__setitem__(0, h)
    mod.get_axon_ntff_profile_hook = lambda: _hook[0]
    sys.modules["antenv.axon_hooks"] = mod
    antenv.axon_hooks = mod
    try:
        from trn_agent_boot.trn_boot import _ntff_profile_via_ctypes

        hook = _ntff_profile_via_ctypes("/opt/axon/libaxon_pjrt.so")
        mod.set_axon_ntff_profile_hook(hook)
    except Exception:
        pass


_ensure_axon_hooks()


def _ensure_axon_hooks():
    """bass_utils' trace path imports antenv.axon_hooks, which this image
    lacks; register a functional stand-in so tracing works when available
    and degrades cleanly otherwise."""
    import types

    if "antenv.axon_hooks" in sys.modules:
        return
    try:
        import antenv
    except ImportError:
        return
    mod = types.ModuleType("antenv.axon_hooks")
    _hook = [None]
    mod.set_axon_ntff_profile_hook = lambda h: _hook.__setitem__(0, h)
    mod.get_axon_ntff_profile_hook = lambda: _hook[0]
    sys.modules["antenv.axon_hooks"] = mod
    antenv.axon_hooks = mod
    try:
        from trn_agent_boot.trn_boot import _ntff_profile_via_ctypes

        hook = _ntff_profile_via_ctypes("/opt/axon/libaxon_pjrt.so")
        mod.set_axon_ntff_profile_hook(hook)
    except Exception:
        pass


_ensure_axon_hooks()


def _ensure_axon_hooks():
    """bass_utils' trace path imports antenv.axon_hooks, which this image
    lacks; register a functional stand-in so tracing works when available
    and degrades cleanly otherwise."""
    import types

    if "antenv.axon_hooks" in sys.modules:
        return
    try:
        import antenv
    except ImportError:
        return
    mod = types.ModuleType("antenv.axon_hooks")
    _hook = [None]
    mod.set_axon_ntff_profile_hook = lambda h: _hook.__setitem__(0, h)
    mod.get_axon_ntff_profile_hook = lambda: _hook[0]
    sys.modules["antenv.axon_hooks"] = mod
    antenv.axon_hooks = mod
    try:
        from trn_agent_boot.trn_boot import _ntff_profile_via_ctypes

        hook = _ntff_profile_via_ctypes("/opt/axon/libaxon_pjrt.so")
        mod.set_axon_ntff_profile_hook(hook)
    except Exception:
        pass


_ensure_axon_hooks()

dt = mybir.dt

HID = 512
HEADS = 8
HD = 64  # head dim
B = 2
S = 4096
QR = 1024  # query rows per core
N_CORES = 8

MM_DTYPE = dt.bfloat16

LAST_RESULT = None  # stash of BassKernelResults for test harnesses


def _split_drain_waits(nc, max_waits=1):
    """neuronxcc CoreV3 codegen rejects instructions carrying more than one
    sem wait; spill extra waits onto preceding InstNoOp on the same engine."""
    n = 0
    for bb in nc.main_func.blocks:
        out = []
        for ins in bb.instructions:
            si = ins.sync_info
            if (
                not isinstance(ins, mybir.InstNoOp)
                and si is not None
                and si.on_wait
                and len(si.on_wait) > max_waits
            ):
                waits = list(si.on_wait)
                for i, w in enumerate(waits[max_waits:]):
                    nop = mybir.InstNoOp(
                        name=f"{ins.name}_wspill{i}",
                        engine=ins.engine,
                        ins=[],
                        outs=[],
                        sync_info=mybir.SyncInfo(on_wait=[w], on_update=[]),
                    )
                    nc.register_instruction(nop, overwrite=True)
                    out.append(nop)
                    n += 1
                ins.sync_info = mybir.SyncInfo(
                    on_wait=waits[:max_waits], on_update=list(si.on_update or [])
                )
            out.append(ins)
        bb.instructions[:] = out
    return n


def build_nc(s=S, qr=QR, mm_dtype=MM_DTYPE):
    f32 = dt.float32
    C = HID // 128  # hidden chunks
    NKC = s // 128  # 128-key chunks
    NKB = s // 512  # 512-key blocks (projection granularity)
    NTB = s // 128  # token chunks for V
    NQB = qr // 512  # query blocks per core
    NHP = HEADS // 2  # head pairs
    LAG = 3  # PV drain lag in kc units

    nc = bass.Bass()
    # token/weight tensors arrive pre-chunked from the host so every DMA has
    # 4KB-contiguous per-partition lines
    qT = nc.dram_tensor("qT", [NQB, 128, C, 512], mm_dtype, kind="ExternalInput")
    ktT = nc.dram_tensor("ktT", [NKB, 128, C, 512], mm_dtype, kind="ExternalInput")
    vtT = nc.dram_tensor("vtT", [NKB, 128, C, 512], mm_dtype, kind="ExternalInput")
    mk = nc.dram_tensor("maskk", [128, NQB, NKC, 512], mm_dtype, kind="ExternalInput")
    qwT = nc.dram_tensor("qwT", [128, C, HID], mm_dtype, kind="ExternalInput")
    kwT = nc.dram_tensor("kwT", [128, C, HID], mm_dtype, kind="ExternalInput")
    vwT = nc.dram_tensor("vwT", [128, C, HID], mm_dtype, kind="ExternalInput")
    owT = nc.dram_tensor("owT", [128, C, HID], mm_dtype, kind="ExternalInput")
    outT = nc.dram_tensor("outT", [HID, qr], f32, kind="ExternalOutput")

    qwT_v = qwT[:, :, :]
    kwT_v = kwT[:, :, :]
    vwT_v = vwT[:, :, :]
    owT_v = owT[:, :, :]

    EXP = mybir.ActivationFunctionType.Exp
    MULT = mybir.AluOpType.mult

    with tile.TileContext(nc) as tc:
        with (
            tc.tile_pool(name="pers", bufs=1) as pers,
            tc.tile_pool(name="mask", bufs=2) as mask_pool,
            tc.tile_pool(name="tok", bufs=4) as tok_pool,
            tc.tile_pool(name="pp", bufs=6) as p_pool,
            tc.tile_pool(name="pmp", bufs=LAG + 2) as pm_pool,
            tc.tile_pool(name="ctxn", bufs=5) as ctxn_pool,
            tc.tile_pool(name="rbp", bufs=2) as rb_pool,
            tc.tile_pool(name="rip", bufs=2) as rinv_pool,
            tc.tile_pool(name="osb", bufs=2) as osb_pool,
            tc.tile_pool(name="sps", bufs=2, space="PSUM") as s_pool,
            tc.tile_pool(name="crp", bufs=3, space="PSUM") as cr_pool,
            tc.tile_pool(name="ops", bufs=1, space="PSUM") as o_pool,
        ):
            KT = pers.tile([128, C, s], mm_dtype)
            QT = pers.tile([128, C, qr], mm_dtype)
            V_sb = pers.tile([128, NTB, NHP, 2, HD + 1], mm_dtype)
            ow_sb = pers.tile([128, C, HID], mm_dtype)
            kw_sb = pers.tile([128, C, HID], mm_dtype)
            vw_sb = pers.tile([128, C, HID], mm_dtype)
            ones_sb = pers.tile([65, HD], mm_dtype)

            # ---------------- prologue ----------------
            # DMA order matters: the sync queue drains in order, so emit the
            # startup-critical-path tensors first; the big mask loads ride the
            # scalar-engine DGE queue instead.
            nc.vector.memset(ones_sb[:], 1.0)
            warm = pers.tile([1, 64], f32)
            nc.scalar.activation(
                warm[:], ones_sb[64:65, :], EXP, scale=1.0
            )
            # ones columns of augmented V (denominator trick)
            nc.vector.memset(V_sb[:, :, :, :, HD : HD + 1], 1.0)

            masks = {}

            def load_mask(qb):
                msk = mask_pool.tile([128, NKC, 512], mm_dtype, tag="mask")
                for lo, hi in ((0, 2), (2, 8), (8, 16), (16, 24), (24, 32)):
                    nc.sync.dma_start(msk[:, lo:hi, :], mk[:, qb, lo:hi, :])
                masks[qb] = msk

            _pjn = [0]
            _defer_ref = [None]

            def _proj_ps():
                _pjn[0] += 1
                dq = _defer_ref[0]
                if _pjn[0] % 2 or (dq is not None and len(dq) > 0):
                    t = o_pool.tile([128, 512], f32, tag="ops", name=f"pjo{_pjn[0]}")
                else:
                    t = cr_pool.tile([128, 512], f32, tag="crp", name=f"pjc{_pjn[0]}")
                return t

            def kproj(kb, m, blk):
                # KT[:, m, kb*512:(kb+1)*512] from staged token block
                ps = _proj_ps()
                for c in range(C):
                    nc.tensor.matmul(
                        ps[:],
                        kw_sb[:, c, m * 128 : (m + 1) * 128],
                        blk[:, c, :],
                        start=(c == 0),
                        stop=(c == C - 1),
                    )
                nc.vector.tensor_copy(KT[:, m, kb * 512 : (kb + 1) * 512], ps[:])

            def vproj(tb, blk, on_act=False):
                j = tb % 4
                ps = _proj_ps()
                for c in range(C):
                    nc.tensor.matmul(
                        ps[:],
                        blk[:, c, j * 128 : (j + 1) * 128],
                        vw_sb[:, c, :],
                        start=(c == 0),
                        stop=(c == C - 1),
                    )
                eng = nc.scalar if on_act else nc.vector
                if on_act:
                    nc.scalar.copy(
                        V_sb[:, tb, :, :, 0:HD],
                        ps[:].rearrange("p (hp hi d) -> p hp hi d", hp=NHP, hi=2),
                    )
                else:
                    nc.vector.tensor_copy(
                        V_sb[:, tb, :, :, 0:HD],
                        ps[:].rearrange("p (hp hi d) -> p hp hi d", hp=NHP, hi=2),
                    )

            def stage_k(kb):
                blk = tok_pool.tile([128, C, 512], mm_dtype, tag="tok")
                nc.sync.dma_start(blk[:], ktT[kb, :, :, :])
                return blk

            def stage_v(tbb):
                blk = tok_pool.tile([128, C, 512], mm_dtype, tag="tok")
                nc.gpsimd.dma_start(blk[:], vtT[tbb, :, :, :])
                return blk

            # Q projection: query-block 0 only up front (block 1 is a job)
            qw_sb = pers.tile([128, C, HID], mm_dtype)
            nc.sync.dma_start(qw_sb[:], qwT_v)
            qtok0 = tok_pool.tile([128, C, 512], mm_dtype, tag="tok")
            nc.sync.dma_start(qtok0[:], qT[0, :, :, :])
            nc.sync.dma_start(kw_sb[:], kwT_v)
            kb0 = stage_k(0)
            kb1 = stage_k(1)
            nc.gpsimd.dma_start(vw_sb[:], vwT_v)
            vb0 = stage_v(0)
            load_mask(0)
            nc.sync.dma_start(ow_sb[:], owT_v)

            def qproj(m, qblk, blk):
                ps = _proj_ps()
                for c in range(C):
                    nc.tensor.matmul(
                        ps[:],
                        qw_sb[:, c, m * 128 : (m + 1) * 128],
                        blk[:, c, :],
                        start=(c == 0),
                        stop=(c == C - 1),
                    )
                nc.vector.tensor_copy(QT[:, m, qblk * 512 : (qblk + 1) * 512], ps[:])

            # warm the PE (HAM) while the prologue DMAs land: ~5us of junk
            # matmuls on zeroed SBUF, result discarded
            nc.vector.memset(QT[:, 0, 0:512], 0.0)
            junk_ps = cr_pool.tile([64, 512], f32, tag="crp", name="junk")
            for _w in range(12):
                nc.tensor.matmul(
                    junk_ps[:],
                    ones_sb[0:64, :],
                    QT[0:64, 0, 0:512],
                    start=True,
                    stop=True,
                )
            qproj(0, 0, qtok0)
            kproj(0, 0, kb0)

            # ---- job queues (streamed projections during qb0) ----
            jobs_by_hp = {0: [], 1: [], 2: [], 3: []}
            jobs_by_hp[0].append((0, "kpre", (1, 0)))
            for tb in range(0, 4):
                jobs_by_hp[0].append((tb + 1, "v", tb))
            for tb in range(4, NTB):
                jobs_by_hp[0].append((tb - 4, "v", tb))
            for kb in range(2, NKB):
                jobs_by_hp[0].append((4 * kb - 9, "k", (kb, 0)))
            jobs_by_hp[0].append((26, "k", (0, 1)))
            jobs_by_hp[0].append((28, "k", (1, 1)))
            for i, m in enumerate((1, 2, 3)):
                jobs_by_hp[0].append((14 + 4 * i, "q0", m))
            for kb in range(2, NKB):
                jobs_by_hp[1].append((4 * kb - 9, "k", (kb, 1)))
            jobs_by_hp[1].append((26, "k", (0, 2)))
            jobs_by_hp[1].append((28, "k", (1, 2)))
            for m in range(C):
                jobs_by_hp[3].append((8 + 2 * m, "q", m))
            for kb in range(2, NKB):
                jobs_by_hp[2].append((4 * kb - 9, "k", (kb, 2)))
            jobs_by_hp[2].append((26, "k", (0, 3)))
            jobs_by_hp[2].append((28, "k", (1, 3)))
            for kb in range(2, NKB):
                jobs_by_hp[3].append((4 * kb - 9, "k", (kb, 3)))
            for hp in jobs_by_hp:
                jobs_by_hp[hp].sort()

            class JobRunner:
                """Issues staging DMAs ~2 jobs ahead of compute; runs up to two
                jobs per pump (second only if its deadline has arrived)."""

                def __init__(self):
                    self.queue = []  # [(deadline, kind, args)]
                    self.staged = deque()  # [(deadline, kind, args, tile)]
                    self.v_tiles = {}  # tbb/"q0"/"q1" -> [tile, refs_left]

                def set_jobs(self, jobs):
                    self.queue = sorted(self.queue + list(jobs))

                def _stage_next(self):
                    dl, kind, args = self.queue.pop(0)
                    if kind == "k":
                        blk = stage_k(args[0])
                    elif kind == "kpre":
                        blk = kb1
                    elif kind == "q":
                        if "q1" not in self.v_tiles:
                            qt1 = tok_pool.tile([128, C, 512], mm_dtype, tag="tok")
                            nc.sync.dma_start(qt1[:], qT[1, :, :, :])
                            self.v_tiles["q1"] = [qt1, C]
                        blk = self.v_tiles["q1"][0]
                    elif kind == "q0":
                        blk = self.v_tiles["q0"][0]
                    else:
                        tbb = args // 4
                        if tbb not in self.v_tiles:
                            nref = sum(
                                1
                                for d, k, a in list(self.queue) + [(dl, kind, args)]
                                if k == "v" and a // 4 == tbb
                            )
                            self.v_tiles[tbb] = [stage_v(tbb), nref]
                        blk = self.v_tiles[tbb][0]
                    self.staged.append((dl, kind, args, blk))

                def _unref(self, key):
                    ent = self.v_tiles[key]
                    ent[1] -= 1
                    if ent[1] == 0:
                        del self.v_tiles[key]

                def _run_one(self):
                    dl, kind, args, blk = self.staged.popleft()
                    if kind in ("k", "kpre"):
                        kproj(args[0], args[1], blk)
                    elif kind == "q":
                        qproj(args, 1, blk)
                        self._unref("q1")
                    elif kind == "q0":
                        qproj(args, 0, blk)
                        self._unref("q0")
                    else:
                        vproj(args, blk, on_act=(args <= 21))
                        self._unref(args // 4)

                def pump(self, kc):
                    while len(self.staged) < 2 and self.queue:
                        self._stage_next()
                    ran = 0
                    while (
                        self.staged
                        and ran < 2
                        and (ran == 0 or self.staged[0][0] <= kc)
                    ):
                        self._run_one()
                        ran += 1
                        while len(self.staged) < 2 and self.queue:
                            self._stage_next()

                def drain(self):
                    while self.staged or self.queue:
                        while len(self.staged) < 2 and self.queue:
                            self._stage_next()
                        self._run_one()

            runner = JobRunner()
            runner.v_tiles[0] = [vb0, 4]
            runner.v_tiles["q0"] = [qtok0, 3]

            # ---------------- main attention loop ----------------
            # Per (qb, hp) unit: 32 kc steps of S-pair -> exp -> mask -> PV.
            # The softmax normalization of unit N and the output projection of
            # each query block run as "deferred" steps interleaved into unit
            # N+1's kc loop, so the PE/DVE FIFOs never stall behind them.
            ctxn_by_qb = {0: [], 1: []}
            deferred = deque()
            _defer_ref[0] = deferred

            def make_norm_steps(ctx, ctxn, split=True):
                rinvs = [None, None]
                rbs = [None, None]

                def recip_st(hi, j):
                    def st():
                        if j == 0:
                            r = rinv_pool.tile(
                                [65, 512], mm_dtype, tag="rip", name=f"ri{hi}"
                            )
                            rinvs[hi] = r
                        with nc.allow_low_precision(reason="1/denom in bf16"):
                            nc.vector.reciprocal(
                                rinvs[hi][64:65, j * 128 : (j + 1) * 128],
                                ctx[hi][HD : HD + 1, j * 128 : (j + 1) * 128],
                            )

                    return st

                def rb_st(hi):
                    def st():
                        rb_ps = o_pool.tile([HD, 512], f32, tag="ops", name=f"rb{hi}")
                        nc.tensor.matmul(
                            rb_ps[:],
                            ones_sb[64:65, :],
                            rinvs[hi][64:65, :],
                            start=True,
                            stop=True,
                        )
                        rb = rb_pool.tile([HD, 512], f32, tag="rbp", name=f"rc{hi}")
                        rbs[hi] = rb
                        nc.vector.tensor_copy(rb[:], rb_ps[:])

                    return st

                def ctxn_st(hi):
                    def st():
                        nc.vector.tensor_tensor(
                            ctxn[64 * hi : 64 * hi + HD, :],
                            ctx[hi][0:HD, :],
                            rbs[hi][:],
                            MULT,
                        )

                    return st

                def recip_full(hi):
                    def st():
                        r = rinv_pool.tile(
                            [65, 512], mm_dtype, tag="rip", name=f"rf{hi}"
                        )
                        rinvs[hi] = r
                        with nc.allow_low_precision(reason="1/denom in bf16"):
                            nc.vector.reciprocal(
                                r[64:65, :], ctx[hi][HD : HD + 1, :]
                            )

                    return st

                if not split:
                    return [
                        recip_full(0),
                        rb_st(0),
                        recip_full(1),
                        ctxn_st(0),
                        rb_st(1),
                        ctxn_st(1),
                    ]
                return [
                    recip_st(0, 0),
                    recip_st(0, 1),
                    recip_st(0, 2),
                    recip_st(0, 3),
                    rb_st(0),
                    ctxn_st(0),
                    recip_st(1, 0),
                    recip_st(1, 1),
                    recip_st(1, 2),
                    recip_st(1, 3),
                    rb_st(1),
                    ctxn_st(1),
                ]

            def make_o_step(qb, m):
                def st():
                    if qb == 1 and m % 2:
                        o_ps = cr_pool.tile(
                            [128, 512], f32, tag="crp", name=f"op{qb}{m}"
                        )
                    else:
                        o_ps = o_pool.tile([128, 512], f32, tag="ops", name=f"op{qb}{m}")
                    for c in range(C):
                        nc.tensor.matmul(
                            o_ps[:],
                            ow_sb[:, c, m * 128 : (m + 1) * 128],
                            ctxn_by_qb[qb][c][:],
                            start=(c == 0),
                            stop=(c == C - 1),
                        )
                    o_sb = osb_pool.tile([128, 512], f32, tag="osb", name=f"ob{qb}{m}")
                    nc.vector.tensor_copy(o_sb[:], o_ps[:])
                    nc.sync.dma_start(
                        outT[m * 128 : (m + 1) * 128, qb * 512 : (qb + 1) * 512],
                        o_sb[:],
                    )

                return st

            for qb in range(NQB):
                for hp in range(NHP):
                    if qb == 0:
                        runner.set_jobs(jobs_by_hp[hp])
                    if qb == 0 and hp == 3:
                        load_mask(1)  # prefetch qb1 mask
                    mask_sb = masks[qb]
                    ctx = [None, None]
                    ctxn = ctxn_pool.tile([128, 512], mm_dtype, tag="ctxn")
                    ctxn_by_qb[qb].append(ctxn)
                    pending = deque()

                    def drain_one():
                        it = pending.popleft()
                        kc = it["kc"]
                        if kc == 0:
                            it["ctx"][0] = cr_pool.tile(
                                [HD + 1, 512], f32, tag="crp", name="ctx0"
                            )
                            it["ctx"][1] = cr_pool.tile(
                                [HD + 1, 512], f32, tag="crp", name="ctx1"
                            )
                        for hi in (0, 1):
                            nc.tensor.matmul(
                                it["ctx"][hi][:],
                                V_sb[:, kc, it["hp"], hi, :],
                                it["pm"][:, hi, :],
                                start=(kc == 0),
                                stop=(kc == NKC - 1),
                            )

                    for kc in range(NKC):
                        if deferred:
                            deferred.popleft()()
                        s_ps = s_pool.tile([128, 2, 512], f32, tag="sps")
                        for hi in (0, 1):
                            po = 64 * hi
                            nc.tensor.matmul(
                                s_ps[:, hi, :],
                                KT[po : po + 64, hp, kc * 128 : (kc + 1) * 128],
                                QT[po : po + 64, hp, qb * 512 : (qb + 1) * 512],
                                start=True,
                                stop=True,
                                tile_position=(po, 0),
                            )
                        p_sb = p_pool.tile([128, 2, 512], mm_dtype, tag="pp")
                        nc.scalar.activation(p_sb[:], s_ps[:], EXP, scale=0.125)
                        pm = pm_pool.tile([128, 2, 512], mm_dtype, tag="pmp")
                        nc.vector.tensor_tensor(
                            pm[:],
                            p_sb[:],
                            mask_sb[:, kc : kc + 1, :].to_broadcast((128, 2, 512)),
                            MULT,
                        )
                        pending.append(dict(pm=pm, kc=kc, hp=hp, ctx=ctx))
                        if qb == 0:
                            runner.pump(kc)
                        if len(pending) > LAG:
                            drain_one()
                    while pending:
                        drain_one()
                    while deferred:
                        deferred.popleft()()
                    last_unit = qb == NQB - 1 and hp == NHP - 1
                    steps = make_norm_steps(ctx, ctxn, split=not last_unit)
                    if last_unit:
                        for st in steps:
                            st()
                    else:
                        deferred.extend(steps)
                        if hp == NHP - 1:
                            for m in range(C):
                                deferred.append(make_o_step(qb, m))
                if qb == 0:
                    runner.drain()
            # final output projection (qb1)
            for m in range(C):
                make_o_step(1, m)()

    _split_drain_waits(nc)
    return nc


_NC_CACHE = {}


def _get_nc():
    key = (S, QR)
    if key not in _NC_CACHE:
        _NC_CACHE[key] = build_nc()
    return _NC_CACHE[key]


def kernel(
    q_tokens,
    k_tokens,
    v_tokens,
    mask,
    q_w,
    q_b,
    k_w,
    k_b,
    v_w,
    v_b,
    o_w,
    o_b,
):
    global LAST_RESULT
    np_mm = ml_dtypes.bfloat16 if MM_DTYPE == dt.bfloat16 else np.float32
    q_tokens = np.asarray(q_tokens, np.float32)
    k_tokens = np.asarray(k_tokens, np.float32)
    v_tokens = np.asarray(v_tokens, np.float32)
    mask = np.asarray(mask)
    ac = np.ascontiguousarray

    def cvt(a):
        return ac(a.astype(np_mm))

    def wchunk(w):
        # [512,512] w.T -> [128, C, 512] so DMA lines are 4KB contiguous
        return cvt(
            np.asarray(w, np.float32).T.reshape(4, 128, 512).transpose(1, 0, 2)
        )

    def tchunk(a, nblk):
        # [512, n] (hidden-major) -> [nblk, 128, C, 512]
        return cvt(a.reshape(4, 128, nblk, 512).transpose(2, 1, 0, 3))

    wmap = {
        "qwT": wchunk(q_w),
        "kwT": wchunk(k_w),
        "vwT": wchunk(v_w),
        "owT": wchunk(o_w),
    }
    maskf = (~mask.astype(bool)).astype(np_mm)  # keep-mask: 1 = keep, 0 = masked
    NKC = S // 128
    NQB = QR // 512
    in_maps = []
    for c in range(N_CORES):
        b, qb = divmod(c, N_CORES // B)
        rows = slice(QR * qb, QR * (qb + 1))
        mkk = maskf[b, 0, rows, :].T.reshape(NKC, 128, NQB, 512).transpose(1, 2, 0, 3)
        in_maps.append(
            {
                "qT": tchunk(q_tokens[b, rows, :].T, NQB),
                "ktT": tchunk(k_tokens[b].T, S // 512),
                "vtT": tchunk(v_tokens[b].T, S // 512),
                "maskk": ac(mkk),
                **wmap,
            }
        )
    nc = _get_nc()
    res = run_bass_kernel_spmd(nc, in_maps, core_ids=list(range(N_CORES)))
    LAST_RESULT = res
    out = np.empty((B, S, HID), np.float32)
    for c in range(N_CORES):
        b, qb = divmod(c, N_CORES // B)
        out[b, QR * qb : QR * (qb + 1), :] = res.results[c]["outT"].T
    out += np.asarray(o_b, np.float32).reshape(1, 1, -1)
    return out

